# revision 23
# baseline (speedup 1.0000x reference)
"""FewShotSegmentation Trainium2 kernel.

Math: for each batch b (one per NeuronCore):
  num[k, c]  = sum_{p: mask[p]==k+1} F[c, p]          (masked pooling, K=16)
  seg[p']    = argmax_k  (num[k,:] . q[:, p']) / ||num[k,:]||
The reference's den (pixel count) and query-norm cancel inside the argmax
(positive per-k / per-p' scales), and the eps clamp never binds at these
magnitudes, so neither is computed.

v3 strategy: the kernel is DMA-bound (32 MiB of fp32 input @ ~358 GB/s
per core = ~94 us floor), so all PE work must hide behind the HBM
stream. fp32 matmuls cost 4 PE-cycles per moving column; float32r
(replicated fp32, near-fp32 precision) costs 1 cycle per moving column
when the moving operand is >= 256 columns. Layout:

  pooling:  transpose F tiles on PE in fp32 (exact, 2 cyc/col), gather
            transposed tiles into S (128p, 32j, 512c) spanning 4
            channel chunks; pool each 512-c half with 32 fp32r matmuls
            (onehot_j.T @ S_j, 512 moving cols each).
  match:    dots_g (16k, 512p') += numT_i.T @ q_i as fp32r (64 matmuls,
            512 moving cols), ACT applies 1/||num|| as a per-partition
            scale, 4 small PE transposes per group -> (128p',16k), DVE
            max/max_index argmax. Query DMA'd in 256-col halves so the
            final group's matmuls start before its full 2 MiB lands.

Walrus in this toolchain allows only ONE sync-wait per lowered
instruction for several instruction structs; _hoist_excess_matmul_waits
post-processes the scheduled module, moving excess waits onto inserted
wait-only event-semaphore instructions.
"""

from contextlib import ExitStack

import numpy as np

import concourse.bass as bass
import concourse.mybir as mybir
import concourse.tile as tile
from concourse import masks
from concourse.bass_utils import run_bass_kernel_spmd

B, C, H, W = 8, 1024, 64, 64
P = H * W          # 4096 pixels
K = 16             # foreground classes
PART = 128
NCH = C // PART    # 8 channel chunks
NPJ = P // PART    # 32 pixel chunks
NG = 8             # query column groups
GW = P // NG       # 512 pixels per group
HGW = GW // 2      # 256-pixel query half-groups
JPG = GW // PART   # 4 pixel chunks per group
HALF = 512         # c-columns per pooling S buffer (4 chunks)
NCHH = 4           # channel chunks per pooling half

F32 = mybir.dt.float32
F32R = mybir.dt.float32r
I32 = mybir.dt.int32
U32 = mybir.dt.uint32


def build_nc():
    nc = bass.Bass(target_bir_lowering=False)

    sf = nc.dram_tensor("sf", [C, P], F32, kind="ExternalInput")
    sm = nc.dram_tensor("sm", [P], I32, kind="ExternalInput")
    qf = nc.dram_tensor("qf", [C, P], F32, kind="ExternalInput")
    seg = nc.dram_tensor("seg", [P], I32, kind="ExternalOutput")

    with ExitStack() as ctx:
        tc = ctx.enter_context(tile.TileContext(nc))
        singles = ctx.enter_context(tc.tile_pool(name="singles", bufs=1))

        identity = singles.tile([PART, PART], F32)
        masks.make_identity(nc, identity[:])

        # classvec[p, k] = k+1 for every partition
        classvec_i = singles.tile([PART, K], I32)
        nc.gpsimd.iota(classvec_i[:], pattern=[[1, K]], base=1, channel_multiplier=0)
        classvec = singles.tile([PART, K], F32)
        nc.vector.tensor_copy(classvec[:], classvec_i[:])

        # one-hot masks: onehot[p, j, k] = (sm[j*128+p] == k+1); built
        # after the first F DMA is issued so F gets the head start. The
        # mask is loaded contiguously (32, 128) and transposed on-chip
        # (strided 4-byte DMAs cost ~us in descriptors/RMW).
        mask_nm_i = singles.tile([NPJ, PART], I32)
        mask_nmf = singles.tile([NPJ, PART], F32)
        mask_pm = singles.tile([PART, NPJ], F32)
        onehot = singles.tile([PART, NPJ, K], F32)
        onehotr = singles.tile([PART, NPJ, K], F32R)

        def build_onehot(misc_ps):
            nc.scalar.dma_start(
                out=mask_nm_i[:], in_=sm.rearrange("(n p) -> n p", p=PART)
            )
            nc.vector.tensor_copy(mask_nmf[:], mask_nm_i[:])
            mtr = misc_ps.tile([PART, NPJ], F32, tag="mtr")
            nc.tensor.transpose(mtr[:], mask_nmf[:], identity[:NPJ, :NPJ])
            nc.vector.tensor_copy(mask_pm[:], mtr[:])
            for j in range(NPJ):
                nc.vector.tensor_scalar(
                    onehot[:, j, :],
                    classvec[:],
                    mask_pm[:, j : j + 1],
                    None,
                    op0=mybir.AluOpType.is_equal,
                )
            # 0/1 values are exact in fp32r; one bulk rounding copy
            nc.vector.tensor_copy(onehotr[:], onehot[:])

        # transposed-F gather buffer: S[p, j, c] spans 4 channel chunks.
        # fp32r: the PSUM->SBUF copies round, and the pooling matmuls then
        # run at 1 cycle/moving-column instead of fp32's 4.
        S = singles.tile([PART, NPJ, HALF], F32R)

        # pooled prototypes: k-major (fp32, feeds norms) and c-major fp32r
        numK = singles.tile([K, C], F32)          # (16, 1024)
        numT = singles.tile([PART, NCH, K], F32R)  # c-major (128,16) per chunk
        inv = singles.tile([K, 1], F32)
        nrm2 = singles.tile([K, NCH], F32)
        nrm = singles.tile([K, 1], F32)
        outt = singles.tile([PART, NPJ], F32)
        seg_sb = singles.tile([NPJ, PART], I32)

        def epi_copy(sel, out, in_):
            if sel % 2 == 0:
                nc.vector.tensor_copy(out, in_)
            else:
                nc.scalar.copy(out, in_)

        def s_copy(sel, out, in_):
            # S-gather copies all on DVE (GpSimd cannot read PSUM; the
            # fp32r->fp32r ACT path is unproven); DVE has slack now that
            # the query rounding moved into the cast-DMA
            nc.vector.tensor_copy(out, in_)

        with (
            tc.tile_pool(name="fpool", bufs=3) as fpool,
            tc.tile_pool(name="qpool", bufs=4) as qpool,
            tc.tile_pool(name="scp", bufs=2) as scpool,
            tc.tile_pool(name="dtsb", bufs=4) as dtsbpool,
            tc.tile_pool(name="m8", bufs=4) as m8pool,
            tc.tile_pool(name="mi", bufs=4) as mipool,
            tc.tile_pool(name="dtr", bufs=1, space=bass.MemorySpace.PSUM) as dtrpool,
            tc.tile_pool(name="mps", bufs=1, space=bass.MemorySpace.PSUM) as misc_ps,
        ):
            def numt_chain(ii):
                # c-major numT + squared sums for one chunk of C
                dtr = dtrpool.tile([PART, K], F32)
                nc.tensor.transpose(
                    dtr[:],
                    numK[:, PART * ii : PART * (ii + 1)],
                    identity[:K, :K],
                )
                epi_copy(ii, numT[:, ii, :], dtr[:])
                sqs = scpool.tile([K, PART], F32, tag="sq")
                nc.scalar.square(sqs[:], numK[:, PART * ii : PART * (ii + 1)])
                nc.vector.reduce_sum(
                    nrm2[:, ii : ii + 1], sqs[:], axis=mybir.AxisListType.X
                )

            # ---------------- pooling phase ----------------
            with (
                tc.tile_pool(name="pst", bufs=3, space=bass.MemorySpace.PSUM) as pspool,
                tc.tile_pool(name="pnum", bufs=1, space=bass.MemorySpace.PSUM) as pnpool,
            ):
                # pool-half accumulators live across the chunk loop so the
                # matmuls can interleave with the final chunk's transposes
                pn = [None, None]

                def pool_bank(h, jb):
                    # 4 fp32r matmuls (512 moving cols) for pixel chunks
                    # jb*4..jb*4+3, accumulating into half-h's PSUM bank
                    if pn[h] is None:
                        pnt = pnpool.tile([K, HALF], F32, tag=f"pn{h}")
                        pn[h] = pnt
                    for t in range(4):
                        j = jb * 4 + t
                        nc.tensor.matmul(
                            pn[h][:],
                            lhsT=onehotr[:, j, :],
                            rhs=S[:, j, :],
                            start=(j == 0),
                            stop=(j == NPJ - 1),
                            skip_group_check=True,
                        )

                for i in range(NCH):
                    F = fpool.tile([PART, P], F32)
                    if i == 0:
                        # split the first chunks so transposes start earlier
                        for q4 in range(4):
                            nc.gpsimd.dma_start(
                                out=F[:, 1024 * q4 : 1024 * (q4 + 1)],
                                in_=sf[:PART, 1024 * q4 : 1024 * (q4 + 1)],
                            )
                    elif i == 1:
                        for q2 in range(2):
                            nc.gpsimd.dma_start(
                                out=F[:, 2048 * q2 : 2048 * (q2 + 1)],
                                in_=sf[PART : 2 * PART, 2048 * q2 : 2048 * (q2 + 1)],
                            )
                    else:
                        nc.gpsimd.dma_start(
                            out=F[:], in_=sf[PART * i : PART * (i + 1), :]
                        )
                    if i == 0:
                        build_onehot(misc_ps)
                    m = i % NCHH
                    for jb in range(NPJ // 4):
                        pst = pspool.tile([PART, 4, PART], F32)
                        for t in range(4):
                            j = jb * 4 + t
                            nc.tensor.transpose(
                                pst[:, t, :],
                                F[:, PART * j : PART * (j + 1)],
                                identity[:],
                            )
                        s_copy(
                            jb,
                            S[:, jb * 4 : jb * 4 + 4, PART * m : PART * (m + 1)],
                            pst[:],
                        )
                        if i == NCHH - 1 or i == NCH - 1:
                            # pool this bank's pixel chunks right behind the
                            # copy: the half finishes ~one bank after the
                            # last transpose instead of 13us later
                            pool_bank(i // NCHH, jb)
                    if i == NCHH - 1:
                        nc.scalar.copy(numK[:, :HALF], pn[0][:])
                        # chunks 0-3 of numT/norms only need pool half A;
                        # keep them off the post-pool-B critical path
                        for ii in range(NCHH):
                            numt_chain(ii)
                    elif i == NCH - 1:
                        nc.scalar.copy(numK[:, HALF:], pn[1][:])
                        for ii in range(NCHH, NCH):
                            numt_chain(ii)
                nc.vector.reduce_sum(nrm[:], nrm2[:], axis=mybir.AxisListType.X)
                nc.scalar.sqrt(nrm[:], nrm[:])
                nc.vector.reciprocal(inv[:], nrm[:])

            # ---------------- match phase ----------------
            with tc.tile_pool(
                name="pdot", bufs=4, space=bass.MemorySpace.PSUM
            ) as pdpool:
              def post_group(g, sck):
                  for t in range(JPG):
                      dtr = dtrpool.tile([PART, K], F32)
                      nc.tensor.transpose(
                          dtr[:],
                          sck[:, PART * t : PART * (t + 1)],
                          identity[:K, :K],
                      )
                      dt = dtsbpool.tile([PART, K], F32)
                      nc.vector.tensor_copy(dt[:], dtr[:])
                      m8 = m8pool.tile([PART, 8], F32)
                      nc.vector.max(m8[:], dt[:])
                      mi = mipool.tile([PART, 8], U32)
                      nc.vector.max_index(mi[:], m8[:], dt[:])
                      j = g * JPG + t
                      nc.vector.tensor_copy(outt[:, j : j + 1], mi[:, 0:1])

              qview = qf.rearrange("(n p) q -> p n q", p=PART)
              pending = None
              for g in range(NG):
                  pd = pdpool.tile([K, GW], F32)
                  # query group arrives pre-rounded via gpsimd cast-DMA;
                  # 512-wide fp32r matmuls (1 cyc/col)
                  Q = qpool.tile([PART, NCH, GW], F32R)
                  nc.gpsimd.dma_start(
                      out=Q[:], in_=qview[:, :, GW * g : GW * (g + 1)]
                  )
                  for i in range(NCH):
                      nc.tensor.matmul(
                          pd[:],
                          lhsT=numT[:, i, :],
                          rhs=Q[:, i, :],
                          start=(i == 0),
                          stop=(i == NCH - 1),
                          skip_group_check=True,
                      )
                  # scale by 1/||num|| (per-partition) while leaving PSUM
                  sck = scpool.tile([K, GW], F32, tag="sck")
                  nc.scalar.mul(sck[:], pd[:], inv[:])
                  if pending is not None:
                      post_group(*pending)
                  pending = (g, sck)
              post_group(*pending)
              # transpose the f32 index results and store seg contiguously
              otr = misc_ps.tile([NPJ, PART], F32, tag="otr")
              nc.tensor.transpose(otr[:], outt[:], identity[:])
              nc.vector.tensor_copy(seg_sb[:], otr[:])

            nc.scalar.dma_start(
                out=seg.rearrange("(n p) -> n p", p=PART), in_=seg_sb[:]
            )

    _hoist_excess_matmul_waits(nc)
    return nc


def _hoist_excess_matmul_waits(nc):
    """walrus allows only one sync-wait per lowered instruction for some
    instruction structs (fp32 matmul LW, pseudo-DMA, ...); hoist extras
    onto wait-only event-semaphore instructions inserted right before
    the instruction on the same queue."""
    n = 0
    for f in nc.m.functions:
        for bb in f.blocks:
            out, changed = [], False
            for ins in bb.instructions:
                w = list(ins.sync_info.on_wait) if ins.sync_info else []
                if len(w) >= 2:
                    for x in w[:-1]:
                        n += 1
                        out.append(
                            mybir.InstEventSemaphore(
                                name=f"I-wh-{n}",
                                engine=ins.engine,
                                ins=[],
                                outs=[],
                                sync_info=mybir.SyncInfo(on_wait=[x], on_update=[]),
                            )
                        )
                    ins.sync_info = mybir.SyncInfo(
                        on_wait=[w[-1]], on_update=list(ins.sync_info.on_update)
                    )
                    changed = True
                out.append(ins)
            if changed:
                bb.instructions = out


_NC_CACHE = None


def _get_nc():
    global _NC_CACHE
    if _NC_CACHE is None:
        _NC_CACHE = build_nc()
    return _NC_CACHE


def run(inputs: dict, trace: bool = False, **kw):
    """Shard over batch, run on 8 cores, gather. Returns (seg, BassKernelResults)."""
    sf = np.ascontiguousarray(inputs["support_features"], dtype=np.float32)
    sm = np.ascontiguousarray(inputs["support_masks"], dtype=np.int32)
    qf = np.ascontiguousarray(inputs["query_features"], dtype=np.float32)
    assert sf.shape == (B, C, H, W), sf.shape
    assert sm.shape == (B, 1, H, W), sm.shape
    assert qf.shape == (B, C, H, W), qf.shape

    in_maps = [
        {
            "sf": sf[b].reshape(C, P),
            "sm": sm[b].reshape(P),
            "qf": qf[b].reshape(C, P),
        }
        for b in range(B)
    ]
    res = run_bass_kernel_spmd(
        _get_nc(), in_maps, core_ids=list(range(B)), trace=trace, **kw
    )
    seg = np.stack([res.results[b]["seg"] for b in range(B)]).reshape(B, H, W)
    return seg.astype(np.int32), res


def kernel(**inputs) -> np.ndarray:
    seg, _ = run(inputs, trace=False)
    return seg
